# revision 3
# baseline (speedup 1.0000x reference)
"""GAT-style message passing (nn_MicroConv) on 8 Trainium2 NeuronCores.

v5: bf16 datapath. Gather table rows are 272B: [fs bf16 x128 | e_src f32
x4], gathered one 128-edge slot per indirect DMA (the HW DGE consumes one
offset per output partition). Per-edge e_dst uses the indicator-transpose
trick in bf16: per slot, transpose the indicator on the PE and multiply by
the window's e_dst rows, accumulating all slots' results into one PSUM
tile per chunk; a single DVE add then forms the logits in f32. Leaky-relu
+ exp, cast back into the bf16 edge rows, and the segment reduction runs
as one bf16 PE matmul per 128-edge slot accumulating in f32 PSUM. Node
transforms (phases B/C) run in bf16 with batched DMA.
"""

import numpy as np
import ml_dtypes

from concourse import bacc, bass, mybir, tile
from concourse.bass import IndirectOffsetOnAxis
from concourse.bass_utils import run_bass_kernel_spmd

# ---------------------------------------------------------------- constants
N_CORES = 8
H = 4          # heads
D = 32         # feats per head
HD = H * D     # 128
TC = HD + H    # 132 live columns in a table row: [fs | e_src]
TCB = HD + 2 * H   # 136 bf16-element row pitch ([fs bf16 | e_src f32])
W_DST = 32     # dst nodes per window (matmul indicator width)
PGROUP = 4     # windows per PSUM tile (4*32 = 128 partitions)
CHUNK = 32     # slots (128-edge tiles) per chunk
TB = 8         # node tiles per load/store batch in phases B/C
NEG_SLOPE = 0.2
SENT_ESRC = -1.0e30
F32 = mybir.dt.float32
BF16 = mybir.dt.bfloat16
I32 = mybir.dt.int32
NP_BF16 = ml_dtypes.bfloat16


def _cdiv(a, b):
    return (a + b - 1) // b


# ---------------------------------------------------------------- host prep
def _prep(feat_src, feat_dst, w_src, w_dst, attn, src_idx, dst_idx, n_cores):
    n_src, d_in = feat_src.shape
    n_dst = feat_dst.shape[0]
    assert d_in % 128 == 0
    fch = d_in // 128

    ndc = _cdiv(n_dst, n_cores)                    # dsts per core
    ndc_pad = _cdiv(ndc, PGROUP * W_DST) * PGROUP * W_DST
    nwin = ndc_pad // W_DST
    nsrc_pad = _cdiv(n_src, 128) * 128
    sent_row = nsrc_pad                            # sentinel table row id
    nt_src = nsrc_pad // 128
    nt_dst = ndc_pad // 128

    # ---- edge sort by dst
    perm = np.argsort(dst_idx, kind="stable")
    ds = dst_idx[perm]
    ss = src_idx[perm]

    counts = np.zeros((n_cores, nwin), np.int64)
    per_core = []
    for c in range(n_cores):
        lo, hi = np.searchsorted(ds, [c * ndc, min((c + 1) * ndc, n_dst)])
        d_loc = (ds[lo:hi] - c * ndc).astype(np.int64)
        s_loc = ss[lo:hi].astype(np.int64)
        win = d_loc // W_DST
        counts[c] = np.bincount(win, minlength=nwin)
        per_core.append((d_loc, s_loc, win))

    order = np.argsort(-counts, axis=1, kind="stable")     # [n_cores, nwin]
    sorted_counts = np.take_along_axis(counts, order, axis=1)
    rank_max = sorted_counts.max(axis=0)                   # [nwin]
    t_r = np.maximum(1, _cdiv(rank_max, 128)).astype(np.int64)  # tiles/slotrank
    slot_base = np.concatenate([[0], np.cumsum(t_r)])
    stot = int(slot_base[-1])
    n_chunks = _cdiv(stot, CHUNK)
    stot_pad = n_chunks * CHUNK

    # schedule shared by all cores: slot -> (window rank, tile, ntiles)
    slot_sched = []
    for r in range(nwin):
        for t in range(int(t_r[r])):
            slot_sched.append((r, t, int(t_r[r])))
    assert len(slot_sched) == stot

    # ---- per-core edge slot arrays
    idxs_h, dloc_h = [], []
    for c in range(n_cores):
        d_loc, s_loc, win = per_core[c]
        e_src_ids = np.full((stot_pad, 128), sent_row, np.int32)
        e_dloc = np.zeros((stot_pad, 128), np.float32)
        if len(d_loc):
            rank = np.empty(nwin, np.int64)
            rank[order[c]] = np.arange(nwin)
            win_start = np.concatenate([[0], np.cumsum(counts[c])[:-1]])
            posw = np.arange(len(d_loc)) - win_start[win]
            r_of = rank[win]
            slot = slot_base[r_of] + posw // 128
            lane = posw % 128
            e_src_ids[slot, lane] = s_loc
            e_dloc[slot, lane] = (d_loc - win * W_DST).astype(np.float32)
        # [n_chunks, 128, CHUNK]: arr[i, p, j] = slot i*CHUNK+j, lane p
        def _pack(a):
            return np.ascontiguousarray(
                a.reshape(n_chunks, CHUNK, 128).transpose(0, 2, 1)
            )
        idxs_h.append(_pack(e_src_ids))
        dloc_h.append(_pack(e_dloc))

    # ---- feature tiles, feature-major contiguous: [fch, 128(f), npad(n)]
    def _tiles(feat, npad):
        f = np.zeros((npad, d_in), np.float32)
        f[: feat.shape[0]] = feat
        return np.ascontiguousarray(
            f.reshape(npad, fch, 128).transpose(1, 2, 0)
        ).astype(NP_BF16)

    def _tiles_arr(f):
        return np.ascontiguousarray(
            f.reshape(f.shape[0], fch, 128).transpose(1, 2, 0)
        ).astype(NP_BF16)

    fsT = _tiles(feat_src, nsrc_pad)
    # feat_dst shard rows permuted into slot (sorted-window) order so the
    # e_dst table comes out slot-ordered with compile-time addresses
    fdT = []
    for c in range(n_cores):
        n_here = min(ndc, n_dst - c * ndc)
        fd_slot = np.zeros((ndc_pad, d_in), np.float32)
        for r in range(nwin):
            w = int(order[c][r])
            d0 = w * W_DST
            n = min(W_DST, n_here - d0)
            if n > 0:
                fd_slot[r * W_DST : r * W_DST + n] = \
                    feat_dst[c * ndc + d0 : c * ndc + d0 + n]
        fdT.append(_tiles_arr(fd_slot))

    # ---- attention selector matrices (pure relayout of attn input)
    a_src = np.zeros((HD, H), np.float32)
    a_dst = np.zeros((HD, H), np.float32)
    for h in range(H):
        a_dst[h * D : (h + 1) * D, h] = attn[h, :D]
        a_src[h * D : (h + 1) * D, h] = attn[h, D:]

    cfg = dict(
        n_src=n_src, n_dst=n_dst, d_in=d_in, fch=fch, ndc=ndc,
        ndc_pad=ndc_pad, nwin=nwin, nsrc_pad=nsrc_pad, sent_row=sent_row,
        nt_src=nt_src, nt_dst=nt_dst, stot=stot, stot_pad=stot_pad,
        n_chunks=n_chunks, slot_sched=slot_sched, n_cores=n_cores,
    )
    common = dict(
        wsrc=np.ascontiguousarray(w_src).astype(NP_BF16),
        wsrcT=np.ascontiguousarray(w_src.T).astype(NP_BF16),
        wdstT=np.ascontiguousarray(w_dst.T).astype(NP_BF16),
        asrc=np.ascontiguousarray(a_src).astype(NP_BF16),
        adst=np.ascontiguousarray(a_dst).astype(NP_BF16),
        fsT=fsT,
    )
    in_maps = []
    for c in range(n_cores):
        m = dict(common)
        m["fdT"] = fdT[c]
        m["idxs"] = idxs_h[c]
        m["dloc"] = dloc_h[c]
        in_maps.append(m)
    return cfg, in_maps, order


# ---------------------------------------------------------------- device kernel
def _build(nc, tc, cfg):
    fch = cfg["fch"]
    d_in = cfg["d_in"]
    nt_src = cfg["nt_src"]
    nt_dst = cfg["nt_dst"]

    # I/O
    fsT = nc.dram_tensor("fsT", [fch, 128, cfg["nsrc_pad"]], BF16,
                         kind="ExternalInput")
    fdT = nc.dram_tensor("fdT", [fch, 128, cfg["ndc_pad"]], BF16,
                         kind="ExternalInput")
    wsrc = nc.dram_tensor("wsrc", [d_in, HD], BF16, kind="ExternalInput")
    wsrcT = nc.dram_tensor("wsrcT", [HD, d_in], BF16, kind="ExternalInput")
    wdstT = nc.dram_tensor("wdstT", [HD, d_in], BF16, kind="ExternalInput")
    asrc = nc.dram_tensor("asrc", [HD, H], BF16, kind="ExternalInput")
    adst = nc.dram_tensor("adst", [HD, H], BF16, kind="ExternalInput")
    idxs = nc.dram_tensor("idxs", [cfg["n_chunks"], 128, CHUNK], I32,
                          kind="ExternalInput")
    dloc = nc.dram_tensor("dloc", [cfg["n_chunks"], 128, CHUNK], F32,
                          kind="ExternalInput")
    out = nc.dram_tensor("out", [cfg["ndc_pad"], HD], F32,
                         kind="ExternalOutput")

    tab = nc.dram_tensor("tab", [cfg["nsrc_pad"] + 128, TCB], BF16,
                         kind="Internal")
    edt = nc.dram_tensor("edt", [cfg["ndc_pad"], H], BF16, kind="Internal")

    import contextlib
    ctx = contextlib.ExitStack()
    with ctx:
        const = ctx.enter_context(tc.tile_pool(name="const", bufs=1))
        sb = ctx.enter_context(tc.tile_pool(name="sb", bufs=3))
        gp = ctx.enter_context(tc.tile_pool(name="gp", bufs=4))
        pp = ctx.enter_context(tc.tile_pool(name="pp", bufs=3, space="PSUM"))

        # ---------------- setup: W_ext = [w_src | M_src], M_dst
        wsT_sb = const.tile([128, d_in], BF16, tag="wsT")
        wdT_sb = const.tile([128, d_in], BF16, tag="wdT")
        asrc_sb = const.tile([128, H], BF16, tag="asrc")
        adst_sb = const.tile([128, H], BF16, tag="adst")
        nc.sync.dma_start(out=wsT_sb[:], in_=wsrcT[:, :])
        nc.sync.dma_start(out=wdT_sb[:], in_=wdstT[:, :])
        nc.sync.dma_start(out=asrc_sb[:], in_=asrc[:, :])
        nc.sync.dma_start(out=adst_sb[:], in_=adst[:, :])

        wext = []
        mdst = []
        for k in range(fch):
            we = const.tile([128, TC], BF16, tag=f"wext{k}")
            nc.sync.dma_start(out=we[:, :HD],
                              in_=wsrc[k * 128 : (k + 1) * 128, :])
            pm = pp.tile([128, 512], F32, tag="acc")
            nc.tensor.matmul(pm[:, :H], wsT_sb[:, k * 128 : (k + 1) * 128],
                             asrc_sb[:], start=True, stop=True)
            nc.vector.tensor_copy(we[:, HD:TC], pm[:, :H])
            wext.append(we)

            md = const.tile([128, H], BF16, tag=f"mdst{k}")
            pm2 = pp.tile([128, 512], F32, tag="acc")
            nc.tensor.matmul(pm2[:, :H], wdT_sb[:, k * 128 : (k + 1) * 128],
                             adst_sb[:], start=True, stop=True)
            nc.vector.tensor_copy(md[:], pm2[:, :H])
            mdst.append(md)

        iota_i = const.tile([128, W_DST], I32, tag="iota_i")
        iota_f = const.tile([128, W_DST], F32, tag="iota_f")
        nc.gpsimd.iota(iota_i[:], pattern=[[1, W_DST]], base=0,
                       channel_multiplier=0)
        nc.vector.tensor_copy(iota_f[:], iota_i[:])

        from concourse.masks import make_identity
        ident = const.tile([128, 128], BF16, tag="ident")
        make_identity(nc, ident[:])

        # ---------------- phase B: e_dst table (slot-rank order)
        for jb in range(0, nt_dst, TB):
            tcur = min(TB, nt_dst - jb)
            lh = sb.tile([128, fch * TB * 128], BF16, tag="lhb")
            lh3 = lh[:].rearrange("p (k m) -> p k m", k=fch)
            for k in range(fch):
                nc.sync.dma_start(
                    out=lh3[:, k, : tcur * 128],
                    in_=fdT[k, :, jb * 128 : (jb + tcur) * 128])
            ebo = sb.tile([128, TB * H], BF16, tag="ebo")
            for t in range(tcur):
                pb = pp.tile([128, 512], F32, tag="acc")
                for k in range(fch):
                    nc.tensor.matmul(
                        pb[:, :H],
                        lh3[:, k, t * 128 : (t + 1) * 128],
                        mdst[k][:], start=(k == 0), stop=(k == fch - 1))
                nc.vector.tensor_copy(ebo[:, t * H : (t + 1) * H], pb[:, :H])
            nc.sync.dma_start(
                out=edt[jb * 128 : (jb + tcur) * 128, :].rearrange(
                    "(t p) c -> p t c", p=128),
                in_=ebo[:].rearrange("p (t c) -> p t c", c=H)[:, :tcur])

        # ---------------- phase C: gather table [fs bf16 | e_src f32]
        for jb in range(0, nt_src, TB):
            tcur = min(TB, nt_src - jb)
            lh = sb.tile([128, fch * TB * 128], BF16, tag="lhc")
            lh3 = lh[:].rearrange("p (k m) -> p k m", k=fch)
            for k in range(fch):
                nc.sync.dma_start(
                    out=lh3[:, k, : tcur * 128],
                    in_=fsT[k, :, jb * 128 : (jb + tcur) * 128])
            tbo = sb.tile([128, TB * TCB], BF16, tag="tbo")
            tbo_f = tbo[:].bitcast(F32).rearrange("p (t c) -> p t c",
                                                  c=TCB // 2)
            tbo_b = tbo[:].rearrange("p (t c) -> p t c", c=TCB)
            for t in range(tcur):
                pc = pp.tile([128, 512], F32, tag="acc")
                for k in range(fch):
                    nc.tensor.matmul(
                        pc[:, :TC],
                        lh3[:, k, t * 128 : (t + 1) * 128],
                        wext[k][:], start=(k == 0), stop=(k == fch - 1))
                nc.vector.tensor_copy(tbo_b[:, t, :HD], pc[:, :HD])
                nc.vector.tensor_copy(tbo_f[:, t, HD // 2 : HD // 2 + H],
                                      pc[:, HD:TC])
            nc.sync.dma_start(
                out=tab[jb * 128 : (jb + tcur) * 128, :].rearrange(
                    "(t p) c -> p t c", p=128),
                in_=tbo_b[:, :tcur])

        # sentinel rows (aligned block of 128): fs = 0, e_src = SENT_ESRC
        st = sb.tile([128, TCB], BF16, tag="sent")
        nc.vector.memset(st[:, :HD], 0.0)
        nc.vector.memset(st[:].bitcast(F32)[:, HD // 2 : HD // 2 + H],
                         SENT_ESRC)
        nc.sync.dma_start(
            out=tab[cfg["nsrc_pad"] : cfg["nsrc_pad"] + 128, :].rearrange(
                "(t p) c -> p t c", p=128),
            in_=st[:].rearrange("p (t c) -> p t c", t=1))

        # preload all chunk indices / window positions once
        n_chunks = cfg["n_chunks"]
        ixs_all = const.tile([128, n_chunks * CHUNK], I32, tag="ixsall")
        dl_all = const.tile([128, n_chunks * CHUNK], F32, tag="dlall")
        nc.sync.dma_start(
            out=ixs_all[:].rearrange("p (i c) -> p i c", c=CHUNK),
            in_=idxs[:, :, :].rearrange("i p c -> p i c"))
        nc.sync.dma_start(
            out=dl_all[:].rearrange("p (i c) -> p i c", c=CHUNK),
            in_=dloc[:, :, :].rearrange("i p c -> p i c"))

        # ---------------- main pass
        sched = cfg["slot_sched"]
        psg = {}
        ewin = {}
        for i in range(cfg["n_chunks"]):
            gt = gp.tile([128, CHUNK * TCB], BF16, tag="gt", bufs=6)
            ixs = ixs_all[:, i * CHUNK : (i + 1) * CHUNK]
            dl = dl_all[:, i * CHUNK : (i + 1) * CHUNK]

            gt3 = gt[:].rearrange("p (s c) -> p s c", c=TCB)
            esv = gt[:].bitcast(F32).rearrange(
                "p (s c) -> p s c", c=TCB // 2)[:, :, HD // 2 : HD // 2 + H]
            # per-slot indirect gathers: one 272B row offset per partition
            for sl in range(CHUNK):
                if i * CHUNK + sl >= cfg["stot"]:
                    break
                nc.gpsimd.indirect_dma_start(
                    out=gt[:, sl * TCB : (sl + 1) * TCB], out_offset=None,
                    in_=tab[:, :],
                    in_offset=IndirectOffsetOnAxis(ap=ixs[:, sl : sl + 1],
                                                   axis=0))

            # indicator S: [128, CHUNK * W_DST] in bf16
            sbt = sb.tile([128, CHUNK * W_DST], BF16, tag="sbt", bufs=6)
            nc.vector.tensor_tensor(
                out=sbt[:].rearrange("p (s w) -> p s w", w=W_DST),
                in0=iota_f[:].rearrange("p (o w) -> p o w", o=1).to_broadcast(
                    [128, CHUNK, W_DST]),
                in1=dl.rearrange("p (s o) -> p s o", o=1).to_broadcast(
                    [128, CHUNK, W_DST]),
                op=mybir.AluOpType.is_equal)

            # per-edge e_dst via PE: transpose indicator, multiply by the
            # window's e_dst rows; all slots accumulate into one PSUM tile
            peb = pp.tile([128, 512], F32, tag="peb", name=f"peb{i}", bufs=3)
            for sl in range(CHUNK):
                s = i * CHUNK + sl
                if s >= cfg["stot"]:
                    break
                r, t, tr = sched[s]
                if t == 0 and r not in ewin:
                    ew = sb.tile([W_DST, H], BF16, tag="ewin", name=f"ew{r}")
                    nc.sync.dma_start(
                        out=ew[:], in_=edt[r * W_DST : (r + 1) * W_DST, :])
                    ewin[r] = ew
                ptr = pp.tile([W_DST, 128], BF16, tag="tr", name=f"ptr{s}",
                              bufs=2)
                nc.tensor.transpose(ptr[:],
                                    sbt[:, sl * W_DST : (sl + 1) * W_DST],
                                    ident[:])
                stx = sb.tile([W_DST, 128], BF16, tag="stx")
                nc.vector.tensor_copy(stx[:], ptr[:])
                nc.tensor.matmul(peb[:, sl * H : (sl + 1) * H], stx[:],
                                 ewin[r][:], start=True, stop=True)
                if t == tr - 1:
                    ewin.pop(r, None)

            # logits: e = e_src + e_dst, leaky-relu, exp (f32, in est)
            ns = min(CHUNK, cfg["stot"] - i * CHUNK)
            est = sb.tile([128, CHUNK * H], F32, tag="est")
            est3 = est[:].rearrange("p (s c) -> p s c", c=H)
            nc.vector.tensor_tensor(
                out=est3[:, :ns], in0=esv[:, :ns],
                in1=peb[:, : ns * H].rearrange("p (s c) -> p s c", c=H),
                op=mybir.AluOpType.add)
            tmp = sb.tile([128, CHUNK * H], F32, tag="tmp")
            nc.vector.tensor_scalar(out=tmp[:, : ns * H],
                                    in0=est[:, : ns * H],
                                    scalar1=NEG_SLOPE,
                                    scalar2=None, op0=mybir.AluOpType.mult)
            nc.vector.tensor_tensor(out=est[:, : ns * H],
                                    in0=est[:, : ns * H],
                                    in1=tmp[:, : ns * H],
                                    op=mybir.AluOpType.max)
            nc.scalar.activation(est[:, : ns * H], est[:, : ns * H],
                                 mybir.ActivationFunctionType.Exp)
            # cast ex back into the bf16 edge rows (cols HD..HD+H)
            nc.vector.tensor_copy(gt3[:, :ns, HD : HD + H], est3[:, :ns])

            # scale fs columns by per-head ex
            for h in range(H):
                fv = gt3[:, :ns, h * D : (h + 1) * D]
                xv = gt3[:, :ns, HD + h : HD + h + 1].to_broadcast(
                    [128, ns, D])
                nc.vector.tensor_tensor(out=fv, in0=fv, in1=xv,
                                        op=mybir.AluOpType.mult)

            # segment matmuls
            for sl in range(CHUNK):
                s = i * CHUNK + sl
                if s >= cfg["stot"]:
                    break
                r, t, tr = sched[s]
                g, q = r // PGROUP, r % PGROUP
                if q == 0 and t == 0:
                    psg[g] = pp.tile([128, 512], F32, tag="acc",
                                     name=f"psg{g}")
                nc.tensor.matmul(
                    psg[g][q * W_DST : (q + 1) * W_DST, :TC],
                    sbt[:, sl * W_DST : (sl + 1) * W_DST],
                    gt[:, sl * TCB : sl * TCB + TC],
                    start=(t == 0), stop=(t == tr - 1),
                    tile_position=(0, q * W_DST))
                if q == PGROUP - 1 and t == tr - 1:
                    # epilogue for group g
                    pt = psg.pop(g)
                    dmx = sb.tile([128, H], F32, tag="dmx")
                    rcp = sb.tile([128, H], F32, tag="rcp")
                    nc.vector.tensor_scalar(out=dmx[:], in0=pt[:, HD:TC],
                                            scalar1=1e-30, scalar2=None,
                                            op0=mybir.AluOpType.max)
                    nc.vector.reciprocal(rcp[:], dmx[:])
                    ot = sb.tile([128, HD], F32, tag="ot")
                    for h in range(H):
                        nc.vector.tensor_scalar(
                            out=ot[:, h * D : (h + 1) * D],
                            in0=pt[:, h * D : (h + 1) * D],
                            scalar1=rcp[:, h : h + 1], scalar2=0.0,
                            op0=mybir.AluOpType.mult,
                            op1=mybir.AluOpType.max)
                    nc.sync.dma_start(
                        out=out[g * 128 : (g + 1) * 128, :], in_=ot[:])
    return out


# ---------------------------------------------------------------- entry point
def kernel(feat_src, feat_dst, w_src, w_dst, attn, src_idx, dst_idx,
           _n_cores=N_CORES, _backend="hw", _results_hook=None,
           _runner=None):
    feat_src = np.asarray(feat_src, np.float32)
    feat_dst = np.asarray(feat_dst, np.float32)
    w_src = np.asarray(w_src, np.float32)
    w_dst = np.asarray(w_dst, np.float32)
    attn = np.asarray(attn, np.float32)
    src_idx = np.asarray(src_idx).astype(np.int32)
    dst_idx = np.asarray(dst_idx).astype(np.int32)

    cfg, in_maps, order = _prep(feat_src, feat_dst, w_src, w_dst, attn,
                                src_idx, dst_idx, _n_cores)

    nc = bacc.Bacc("TRN2", target_bir_lowering=False, debug=False)
    with tile.TileContext(nc) as tc:
        _build(nc, tc, cfg)
    nc.compile()

    if _backend == "sim":
        from concourse.bass_interp import CoreSim
        results = []
        for c in range(_n_cores):
            sim = CoreSim(nc, trace=False, require_nnan=False,
                          require_finite=False)
            for name, arr in in_maps[c].items():
                sim.tensor(name)[:] = arr
            sim.simulate(check_with_hw=False)
            results.append({"out": np.array(sim.tensor("out"))})
        res_obj = None
    elif _runner is not None:
        results = _runner(nc, in_maps)
        res_obj = None
    else:
        res_obj = run_bass_kernel_spmd(nc, in_maps,
                                       core_ids=list(range(_n_cores)))
        results = res_obj.results
    if _results_hook is not None:
        _results_hook(res_obj)

    # unpermute slot-ordered outputs back to dst ids
    n_dst = cfg["n_dst"]
    ndc = cfg["ndc"]
    out_full = np.zeros((n_dst, HD), np.float32)
    for c in range(_n_cores):
        oc = results[c]["out"].reshape(cfg["nwin"], W_DST, HD)
        n_here = min(ndc, n_dst - c * ndc)
        for r in range(cfg["nwin"]):
            w = int(order[c][r])
            d0 = w * W_DST
            n = min(W_DST, n_here - d0)
            if n > 0:
                out_full[c * ndc + d0 : c * ndc + d0 + n] = oc[r, :n]
    return out_full


# revision 4
# speedup vs baseline: 3.6753x; 3.6753x over previous
"""GAT-style message passing (nn_MicroConv) on 8 Trainium2 NeuronCores.

Strategy (dst-node partition): each core owns N_DST/8 destination nodes
and all edges into them. The host sorts edges by dst, buckets them into
32-dst windows, rank-sorts windows by size so all 8 cores share one
compiled schedule, and pads 128-edge slots with sentinel edges.

Device datapath (bf16):
  - Phases B/C build, per core, a replicated gather table
    [N_src+1, 272B] = [fs bf16 x128 | e_src f32 x4] and a local e_dst
    table, via bf16 node-transform matmuls with batched DMA.
  - Main pass: one indirect DMA per 128-edge slot gathers the 272B rows
    (the HW DGE consumes exactly one row offset per output partition).
    Per-edge e_dst uses the indicator-transpose trick: per slot, the PE
    transposes the dst-indicator and multiplies by the window's e_dst
    rows, all slots of a chunk accumulating into one PSUM tile; a single
    DVE add then forms the logits in f32. Leaky-relu + exp, cast back
    into the bf16 edge rows, per-head scaling, and the segment reduction
    runs as one bf16 PE matmul per slot accumulating in f32 PSUM (4
    windows col-tiled per PSUM bank).
  - Epilogue: reciprocal of the denominator columns, scale + ReLU, DMA
    out in slot order; the host unpermutes rows to original dst ids.
"""

import numpy as np
import ml_dtypes

from concourse import bacc, bass, mybir, tile
from concourse.bass import IndirectOffsetOnAxis
from concourse.bass_utils import run_bass_kernel_spmd

# ---------------------------------------------------------------- constants
N_CORES = 8
H = 4          # heads
D = 32         # feats per head
HD = H * D     # 128
TC = HD + H    # 132 live columns in a table row: [fs | e_src]
TCB = HD + 2 * H   # 136 bf16-element row pitch ([fs bf16 | e_src f32])
W_DST = 32     # dst nodes per window (matmul indicator width)
PGROUP = 4     # windows per PSUM tile (4*32 = 128 partitions)
CHUNK = 32     # slots (128-edge tiles) per chunk
TB = 8         # node tiles per load/store batch in phases B/C
NEG_SLOPE = 0.2
SENT_ESRC = -1.0e30
F32 = mybir.dt.float32
BF16 = mybir.dt.bfloat16
I32 = mybir.dt.int32
NP_BF16 = ml_dtypes.bfloat16


def _cdiv(a, b):
    return (a + b - 1) // b


# ---------------------------------------------------------------- host prep
def _prep(feat_src, feat_dst, w_src, w_dst, attn, src_idx, dst_idx, n_cores):
    n_src, d_in = feat_src.shape
    n_dst = feat_dst.shape[0]
    assert d_in % 128 == 0
    fch = d_in // 128

    ndc = _cdiv(n_dst, n_cores)                    # dsts per core
    ndc_pad = _cdiv(ndc, PGROUP * W_DST) * PGROUP * W_DST
    nwin = ndc_pad // W_DST
    nsrc_pad = _cdiv(n_src, 128) * 128
    sent_row = nsrc_pad                            # sentinel table row id
    nt_src = nsrc_pad // 128
    nt_dst = ndc_pad // 128

    # ---- edge sort by dst
    perm = np.argsort(dst_idx, kind="stable")
    ds = dst_idx[perm]
    ss = src_idx[perm]

    counts = np.zeros((n_cores, nwin), np.int64)
    per_core = []
    for c in range(n_cores):
        lo, hi = np.searchsorted(ds, [c * ndc, min((c + 1) * ndc, n_dst)])
        d_loc = (ds[lo:hi] - c * ndc).astype(np.int64)
        s_loc = ss[lo:hi].astype(np.int64)
        win = d_loc // W_DST
        counts[c] = np.bincount(win, minlength=nwin)
        per_core.append((d_loc, s_loc, win))

    order = np.argsort(-counts, axis=1, kind="stable")     # [n_cores, nwin]
    sorted_counts = np.take_along_axis(counts, order, axis=1)
    rank_max = sorted_counts.max(axis=0)                   # [nwin]
    t_r = np.maximum(1, _cdiv(rank_max, 128)).astype(np.int64)  # tiles/slotrank
    slot_base = np.concatenate([[0], np.cumsum(t_r)])
    stot = int(slot_base[-1])
    n_chunks = _cdiv(stot, CHUNK)
    stot_pad = n_chunks * CHUNK

    # schedule shared by all cores: slot -> (window rank, tile, ntiles)
    slot_sched = []
    for r in range(nwin):
        for t in range(int(t_r[r])):
            slot_sched.append((r, t, int(t_r[r])))
    assert len(slot_sched) == stot

    # ---- per-core edge slot arrays
    idxs_h, dloc_h = [], []
    for c in range(n_cores):
        d_loc, s_loc, win = per_core[c]
        e_src_ids = np.full((stot_pad, 128), sent_row, np.int32)
        e_dloc = np.zeros((stot_pad, 128), np.float32)
        if len(d_loc):
            rank = np.empty(nwin, np.int64)
            rank[order[c]] = np.arange(nwin)
            win_start = np.concatenate([[0], np.cumsum(counts[c])[:-1]])
            posw = np.arange(len(d_loc)) - win_start[win]
            r_of = rank[win]
            slot = slot_base[r_of] + posw // 128
            lane = posw % 128
            e_src_ids[slot, lane] = s_loc
            e_dloc[slot, lane] = (d_loc - win * W_DST).astype(np.float32)
        # [n_chunks, 128, CHUNK]: arr[i, p, j] = slot i*CHUNK+j, lane p
        def _pack(a):
            return np.ascontiguousarray(
                a.reshape(n_chunks, CHUNK, 128).transpose(0, 2, 1)
            )
        idxs_h.append(_pack(e_src_ids))
        dloc_h.append(_pack(e_dloc))

    # ---- feature tiles, feature-major contiguous: [fch, 128(f), npad(n)]
    def _tiles(feat, npad):
        f = np.zeros((npad, d_in), np.float32)
        f[: feat.shape[0]] = feat
        return np.ascontiguousarray(
            f.reshape(npad, fch, 128).transpose(1, 2, 0)
        ).astype(NP_BF16)

    def _tiles_arr(f):
        return np.ascontiguousarray(
            f.reshape(f.shape[0], fch, 128).transpose(1, 2, 0)
        ).astype(NP_BF16)

    fsT = _tiles(feat_src, nsrc_pad)
    # feat_dst shard rows permuted into slot (sorted-window) order so the
    # e_dst table comes out slot-ordered with compile-time addresses
    fdT = []
    for c in range(n_cores):
        n_here = min(ndc, n_dst - c * ndc)
        fd_slot = np.zeros((ndc_pad, d_in), np.float32)
        for r in range(nwin):
            w = int(order[c][r])
            d0 = w * W_DST
            n = min(W_DST, n_here - d0)
            if n > 0:
                fd_slot[r * W_DST : r * W_DST + n] = \
                    feat_dst[c * ndc + d0 : c * ndc + d0 + n]
        fdT.append(_tiles_arr(fd_slot))

    # ---- attention selector matrices (pure relayout of attn input)
    a_src = np.zeros((HD, H), np.float32)
    a_dst = np.zeros((HD, H), np.float32)
    for h in range(H):
        a_dst[h * D : (h + 1) * D, h] = attn[h, :D]
        a_src[h * D : (h + 1) * D, h] = attn[h, D:]

    cfg = dict(
        n_src=n_src, n_dst=n_dst, d_in=d_in, fch=fch, ndc=ndc,
        ndc_pad=ndc_pad, nwin=nwin, nsrc_pad=nsrc_pad, sent_row=sent_row,
        nt_src=nt_src, nt_dst=nt_dst, stot=stot, stot_pad=stot_pad,
        n_chunks=n_chunks, slot_sched=slot_sched, n_cores=n_cores,
    )
    common = dict(
        wsrc=np.ascontiguousarray(w_src).astype(NP_BF16),
        wsrcT=np.ascontiguousarray(w_src.T).astype(NP_BF16),
        wdstT=np.ascontiguousarray(w_dst.T).astype(NP_BF16),
        asrc=np.ascontiguousarray(a_src).astype(NP_BF16),
        adst=np.ascontiguousarray(a_dst).astype(NP_BF16),
        fsT=fsT,
    )
    in_maps = []
    for c in range(n_cores):
        m = dict(common)
        m["fdT"] = fdT[c]
        m["idxs"] = idxs_h[c]
        m["dloc"] = dloc_h[c]
        in_maps.append(m)
    return cfg, in_maps, order


# ---------------------------------------------------------------- device kernel
def _build(nc, tc, cfg):
    fch = cfg["fch"]
    d_in = cfg["d_in"]
    nt_src = cfg["nt_src"]
    nt_dst = cfg["nt_dst"]

    # I/O
    fsT = nc.dram_tensor("fsT", [fch, 128, cfg["nsrc_pad"]], BF16,
                         kind="ExternalInput")
    fdT = nc.dram_tensor("fdT", [fch, 128, cfg["ndc_pad"]], BF16,
                         kind="ExternalInput")
    wsrc = nc.dram_tensor("wsrc", [d_in, HD], BF16, kind="ExternalInput")
    wsrcT = nc.dram_tensor("wsrcT", [HD, d_in], BF16, kind="ExternalInput")
    wdstT = nc.dram_tensor("wdstT", [HD, d_in], BF16, kind="ExternalInput")
    asrc = nc.dram_tensor("asrc", [HD, H], BF16, kind="ExternalInput")
    adst = nc.dram_tensor("adst", [HD, H], BF16, kind="ExternalInput")
    idxs = nc.dram_tensor("idxs", [cfg["n_chunks"], 128, CHUNK], I32,
                          kind="ExternalInput")
    dloc = nc.dram_tensor("dloc", [cfg["n_chunks"], 128, CHUNK], F32,
                          kind="ExternalInput")
    out = nc.dram_tensor("out", [cfg["ndc_pad"], HD], F32,
                         kind="ExternalOutput")

    tab = nc.dram_tensor("tab", [cfg["nsrc_pad"] + 128, TCB], BF16,
                         kind="Internal")
    edt = nc.dram_tensor("edt", [cfg["ndc_pad"], H], BF16, kind="Internal")

    import contextlib
    ctx = contextlib.ExitStack()
    with ctx:
        const = ctx.enter_context(tc.tile_pool(name="const", bufs=1))
        sb = ctx.enter_context(tc.tile_pool(name="sb", bufs=3))
        gp = ctx.enter_context(tc.tile_pool(name="gp", bufs=4))
        pp = ctx.enter_context(tc.tile_pool(name="pp", bufs=3, space="PSUM"))

        # ---------------- setup: W_ext = [w_src | M_src], M_dst
        wsT_sb = const.tile([128, d_in], BF16, tag="wsT")
        wdT_sb = const.tile([128, d_in], BF16, tag="wdT")
        asrc_sb = const.tile([128, H], BF16, tag="asrc")
        adst_sb = const.tile([128, H], BF16, tag="adst")
        nc.sync.dma_start(out=wsT_sb[:], in_=wsrcT[:, :])
        nc.sync.dma_start(out=wdT_sb[:], in_=wdstT[:, :])
        nc.sync.dma_start(out=asrc_sb[:], in_=asrc[:, :])
        nc.sync.dma_start(out=adst_sb[:], in_=adst[:, :])

        wext = []
        mdst = []
        for k in range(fch):
            we = const.tile([128, TC], BF16, tag=f"wext{k}")
            nc.sync.dma_start(out=we[:, :HD],
                              in_=wsrc[k * 128 : (k + 1) * 128, :])
            pm = pp.tile([128, 512], F32, tag="acc")
            nc.tensor.matmul(pm[:, :H], wsT_sb[:, k * 128 : (k + 1) * 128],
                             asrc_sb[:], start=True, stop=True)
            nc.vector.tensor_copy(we[:, HD:TC], pm[:, :H])
            wext.append(we)

            md = const.tile([128, H], BF16, tag=f"mdst{k}")
            pm2 = pp.tile([128, 512], F32, tag="acc")
            nc.tensor.matmul(pm2[:, :H], wdT_sb[:, k * 128 : (k + 1) * 128],
                             adst_sb[:], start=True, stop=True)
            nc.vector.tensor_copy(md[:], pm2[:, :H])
            mdst.append(md)

        iota_i = const.tile([128, W_DST], I32, tag="iota_i")
        iota_f = const.tile([128, W_DST], F32, tag="iota_f")
        nc.gpsimd.iota(iota_i[:], pattern=[[1, W_DST]], base=0,
                       channel_multiplier=0)
        nc.vector.tensor_copy(iota_f[:], iota_i[:])

        from concourse.masks import make_identity
        ident = const.tile([128, 128], BF16, tag="ident")
        make_identity(nc, ident[:])

        # ---------------- phase B: e_dst table (slot-rank order)
        for jb in range(0, nt_dst, TB):
            tcur = min(TB, nt_dst - jb)
            lh = sb.tile([128, fch * TB * 128], BF16, tag="lhb")
            lh3 = lh[:].rearrange("p (k m) -> p k m", k=fch)
            for k in range(fch):
                nc.sync.dma_start(
                    out=lh3[:, k, : tcur * 128],
                    in_=fdT[k, :, jb * 128 : (jb + tcur) * 128])
            ebo = sb.tile([128, TB * H], BF16, tag="ebo")
            for t in range(tcur):
                pb = pp.tile([128, 512], F32, tag="acc")
                for k in range(fch):
                    nc.tensor.matmul(
                        pb[:, :H],
                        lh3[:, k, t * 128 : (t + 1) * 128],
                        mdst[k][:], start=(k == 0), stop=(k == fch - 1))
                nc.vector.tensor_copy(ebo[:, t * H : (t + 1) * H], pb[:, :H])
            nc.sync.dma_start(
                out=edt[jb * 128 : (jb + tcur) * 128, :].rearrange(
                    "(t p) c -> p t c", p=128),
                in_=ebo[:].rearrange("p (t c) -> p t c", c=H)[:, :tcur])

        # ---------------- phase C: gather table [fs bf16 | e_src f32]
        for jb in range(0, nt_src, TB):
            tcur = min(TB, nt_src - jb)
            lh = sb.tile([128, fch * TB * 128], BF16, tag="lhc")
            lh3 = lh[:].rearrange("p (k m) -> p k m", k=fch)
            for k in range(fch):
                nc.sync.dma_start(
                    out=lh3[:, k, : tcur * 128],
                    in_=fsT[k, :, jb * 128 : (jb + tcur) * 128])
            tbo = sb.tile([128, TB * TCB], BF16, tag="tbo")
            tbo_f = tbo[:].bitcast(F32).rearrange("p (t c) -> p t c",
                                                  c=TCB // 2)
            tbo_b = tbo[:].rearrange("p (t c) -> p t c", c=TCB)
            for t in range(tcur):
                pc = pp.tile([128, 512], F32, tag="acc")
                for k in range(fch):
                    nc.tensor.matmul(
                        pc[:, :TC],
                        lh3[:, k, t * 128 : (t + 1) * 128],
                        wext[k][:], start=(k == 0), stop=(k == fch - 1))
                nc.vector.tensor_copy(tbo_b[:, t, :HD], pc[:, :HD])
                nc.vector.tensor_copy(tbo_f[:, t, HD // 2 : HD // 2 + H],
                                      pc[:, HD:TC])
            nc.sync.dma_start(
                out=tab[jb * 128 : (jb + tcur) * 128, :].rearrange(
                    "(t p) c -> p t c", p=128),
                in_=tbo_b[:, :tcur])

        # sentinel rows (aligned block of 128): fs = 0, e_src = SENT_ESRC
        st = sb.tile([128, TCB], BF16, tag="sent")
        nc.vector.memset(st[:, :HD], 0.0)
        nc.vector.memset(st[:].bitcast(F32)[:, HD // 2 : HD // 2 + H],
                         SENT_ESRC)
        nc.sync.dma_start(
            out=tab[cfg["nsrc_pad"] : cfg["nsrc_pad"] + 128, :].rearrange(
                "(t p) c -> p t c", p=128),
            in_=st[:].rearrange("p (t c) -> p t c", t=1))

        # preload all chunk indices / window positions once
        n_chunks = cfg["n_chunks"]
        ixs_all = const.tile([128, n_chunks * CHUNK], I32, tag="ixsall")
        dl_all = const.tile([128, n_chunks * CHUNK], F32, tag="dlall")
        nc.sync.dma_start(
            out=ixs_all[:].rearrange("p (i c) -> p i c", c=CHUNK),
            in_=idxs[:, :, :].rearrange("i p c -> p i c"))
        nc.sync.dma_start(
            out=dl_all[:].rearrange("p (i c) -> p i c", c=CHUNK),
            in_=dloc[:, :, :].rearrange("i p c -> p i c"))

        # ---------------- main pass
        sched = cfg["slot_sched"]
        psg = {}
        ewin = {}
        for i in range(cfg["n_chunks"]):
            gt = gp.tile([128, CHUNK * TCB], BF16, tag="gt", bufs=6)
            ixs = ixs_all[:, i * CHUNK : (i + 1) * CHUNK]
            dl = dl_all[:, i * CHUNK : (i + 1) * CHUNK]

            gt3 = gt[:].rearrange("p (s c) -> p s c", c=TCB)
            esv = gt[:].bitcast(F32).rearrange(
                "p (s c) -> p s c", c=TCB // 2)[:, :, HD // 2 : HD // 2 + H]
            # per-slot indirect gathers: one 272B row offset per partition
            for sl in range(CHUNK):
                if i * CHUNK + sl >= cfg["stot"]:
                    break
                nc.gpsimd.indirect_dma_start(
                    out=gt[:, sl * TCB : (sl + 1) * TCB], out_offset=None,
                    in_=tab[:, :],
                    in_offset=IndirectOffsetOnAxis(ap=ixs[:, sl : sl + 1],
                                                   axis=0))

            # indicator S: [128, CHUNK * W_DST] in bf16
            sbt = sb.tile([128, CHUNK * W_DST], BF16, tag="sbt", bufs=6)
            nc.vector.tensor_tensor(
                out=sbt[:].rearrange("p (s w) -> p s w", w=W_DST),
                in0=iota_f[:].rearrange("p (o w) -> p o w", o=1).to_broadcast(
                    [128, CHUNK, W_DST]),
                in1=dl.rearrange("p (s o) -> p s o", o=1).to_broadcast(
                    [128, CHUNK, W_DST]),
                op=mybir.AluOpType.is_equal)

            # per-edge e_dst via PE: transpose indicator, multiply by the
            # window's e_dst rows; all slots accumulate into one PSUM tile
            peb = pp.tile([128, 512], F32, tag="peb", name=f"peb{i}", bufs=3)
            for sl in range(CHUNK):
                s = i * CHUNK + sl
                if s >= cfg["stot"]:
                    break
                r, t, tr = sched[s]
                if t == 0 and r not in ewin:
                    ew = sb.tile([W_DST, H], BF16, tag="ewin", name=f"ew{r}")
                    nc.sync.dma_start(
                        out=ew[:], in_=edt[r * W_DST : (r + 1) * W_DST, :])
                    ewin[r] = ew
                ptr = pp.tile([W_DST, 128], BF16, tag="tr", name=f"ptr{s}",
                              bufs=2)
                nc.tensor.transpose(ptr[:],
                                    sbt[:, sl * W_DST : (sl + 1) * W_DST],
                                    ident[:])
                stx = sb.tile([W_DST, 128], BF16, tag="stx")
                nc.vector.tensor_copy(stx[:], ptr[:])
                nc.tensor.matmul(peb[:, sl * H : (sl + 1) * H], stx[:],
                                 ewin[r][:], start=True, stop=True)
                if t == tr - 1:
                    ewin.pop(r, None)

            # logits: e = e_src + e_dst, leaky-relu, exp (f32, in est)
            ns = min(CHUNK, cfg["stot"] - i * CHUNK)
            est = sb.tile([128, CHUNK * H], F32, tag="est")
            est3 = est[:].rearrange("p (s c) -> p s c", c=H)
            nc.vector.tensor_tensor(
                out=est3[:, :ns], in0=esv[:, :ns],
                in1=peb[:, : ns * H].rearrange("p (s c) -> p s c", c=H),
                op=mybir.AluOpType.add)
            tmp = sb.tile([128, CHUNK * H], F32, tag="tmp")
            nc.vector.tensor_scalar(out=tmp[:, : ns * H],
                                    in0=est[:, : ns * H],
                                    scalar1=NEG_SLOPE,
                                    scalar2=None, op0=mybir.AluOpType.mult)
            nc.vector.tensor_tensor(out=est[:, : ns * H],
                                    in0=est[:, : ns * H],
                                    in1=tmp[:, : ns * H],
                                    op=mybir.AluOpType.max)
            nc.scalar.activation(est[:, : ns * H], est[:, : ns * H],
                                 mybir.ActivationFunctionType.Exp)
            # cast ex back into the bf16 edge rows (cols HD..HD+H)
            nc.vector.tensor_copy(gt3[:, :ns, HD : HD + H], est3[:, :ns])

            # scale fs columns by per-head ex
            for h in range(H):
                fv = gt3[:, :ns, h * D : (h + 1) * D]
                xv = gt3[:, :ns, HD + h : HD + h + 1].to_broadcast(
                    [128, ns, D])
                nc.vector.tensor_tensor(out=fv, in0=fv, in1=xv,
                                        op=mybir.AluOpType.mult)

            # segment matmuls
            for sl in range(CHUNK):
                s = i * CHUNK + sl
                if s >= cfg["stot"]:
                    break
                r, t, tr = sched[s]
                g, q = r // PGROUP, r % PGROUP
                if q == 0 and t == 0:
                    psg[g] = pp.tile([128, 512], F32, tag="acc",
                                     name=f"psg{g}")
                nc.tensor.matmul(
                    psg[g][q * W_DST : (q + 1) * W_DST, :TC],
                    sbt[:, sl * W_DST : (sl + 1) * W_DST],
                    gt[:, sl * TCB : sl * TCB + TC],
                    start=(t == 0), stop=(t == tr - 1),
                    tile_position=(0, q * W_DST))
                if q == PGROUP - 1 and t == tr - 1:
                    # epilogue for group g
                    pt = psg.pop(g)
                    dmx = sb.tile([128, H], F32, tag="dmx")
                    rcp = sb.tile([128, H], F32, tag="rcp")
                    nc.vector.tensor_scalar(out=dmx[:], in0=pt[:, HD:TC],
                                            scalar1=1e-30, scalar2=None,
                                            op0=mybir.AluOpType.max)
                    nc.vector.reciprocal(rcp[:], dmx[:])
                    ot = sb.tile([128, HD], F32, tag="ot")
                    for h in range(H):
                        nc.vector.tensor_scalar(
                            out=ot[:, h * D : (h + 1) * D],
                            in0=pt[:, h * D : (h + 1) * D],
                            scalar1=rcp[:, h : h + 1], scalar2=0.0,
                            op0=mybir.AluOpType.mult,
                            op1=mybir.AluOpType.max)
                    nc.sync.dma_start(
                        out=out[g * 128 : (g + 1) * 128, :], in_=ot[:])
    return out


# ---------------------------------------------------------------- entry point
def kernel(feat_src, feat_dst, w_src, w_dst, attn, src_idx, dst_idx,
           _n_cores=N_CORES, _backend="hw", _results_hook=None,
           _runner=None):
    feat_src = np.asarray(feat_src, np.float32)
    feat_dst = np.asarray(feat_dst, np.float32)
    w_src = np.asarray(w_src, np.float32)
    w_dst = np.asarray(w_dst, np.float32)
    attn = np.asarray(attn, np.float32)
    src_idx = np.asarray(src_idx).astype(np.int32)
    dst_idx = np.asarray(dst_idx).astype(np.int32)

    cfg, in_maps, order = _prep(feat_src, feat_dst, w_src, w_dst, attn,
                                src_idx, dst_idx, _n_cores)

    nc = bacc.Bacc("TRN2", target_bir_lowering=False, debug=False)
    with tile.TileContext(nc) as tc:
        _build(nc, tc, cfg)
    nc.compile()

    if _backend == "sim":
        from concourse.bass_interp import CoreSim
        results = []
        for c in range(_n_cores):
            sim = CoreSim(nc, trace=False, require_nnan=False,
                          require_finite=False)
            for name, arr in in_maps[c].items():
                sim.tensor(name)[:] = arr
            sim.simulate(check_with_hw=False)
            results.append({"out": np.array(sim.tensor("out"))})
        res_obj = None
    elif _runner is not None:
        results = _runner(nc, in_maps)
        res_obj = None
    else:
        res_obj = run_bass_kernel_spmd(nc, in_maps,
                                       core_ids=list(range(_n_cores)))
        results = res_obj.results
    if _results_hook is not None:
        _results_hook(res_obj)

    # unpermute slot-ordered outputs back to dst ids
    n_dst = cfg["n_dst"]
    ndc = cfg["ndc"]
    out_full = np.zeros((n_dst, HD), np.float32)
    for c in range(_n_cores):
        oc = results[c]["out"].reshape(cfg["nwin"], W_DST, HD)
        n_here = min(ndc, n_dst - c * ndc)
        for r in range(cfg["nwin"]):
            w = int(order[c][r])
            d0 = w * W_DST
            n = min(W_DST, n_here - d0)
            if n > 0:
                out_full[c * ndc + d0 : c * ndc + d0 + n] = oc[r, :n]
    return out_full


# revision 5
# speedup vs baseline: 4.0192x; 1.0936x over previous
"""GAT-style message passing (nn_MicroConv) on 8 Trainium2 NeuronCores.

v5: bf16 datapath. Gather table rows are 272B: [fs bf16 x128 | e_src f32
x4], gathered one 128-edge slot per indirect DMA (the HW DGE consumes one
offset per output partition). Per-edge e_dst uses the indicator-transpose
trick in bf16: per slot, transpose the indicator on the PE and multiply by
the window's e_dst rows, accumulating all slots' results into one PSUM
tile per chunk; a single DVE add then forms the logits in f32. Leaky-relu
+ exp, cast back into the bf16 edge rows, and the segment reduction runs
as one bf16 PE matmul per 128-edge slot accumulating in f32 PSUM. Node
transforms (phases B/C) run in bf16 with batched DMA.
"""

import numpy as np
import ml_dtypes

from concourse import bacc, bass, mybir, tile
from concourse.bass import IndirectOffsetOnAxis
from concourse.bass_utils import run_bass_kernel_spmd

# ---------------------------------------------------------------- constants
N_CORES = 8
H = 4          # heads
D = 32         # feats per head
HD = H * D     # 128
TC = HD + H    # 132 live columns in a table row: [fs | e_src]
TCB = HD + 2 * H   # 136 bf16-element row pitch ([fs bf16 | e_src f32])
W_DST = 32     # dst nodes per window (matmul indicator width)
PGROUP = 4     # windows per PSUM tile (4*32 = 128 partitions)
CHUNK = 32     # slots (128-edge tiles) per chunk
TB = 8         # node tiles per load/store batch in phases B/C
NEG_SLOPE = 0.2
SENT_ESRC = -1.0e30
F32 = mybir.dt.float32
BF16 = mybir.dt.bfloat16
I32 = mybir.dt.int32
NP_BF16 = ml_dtypes.bfloat16


def _cdiv(a, b):
    return (a + b - 1) // b


# ---------------------------------------------------------------- host prep
def _prep(feat_src, feat_dst, w_src, w_dst, attn, src_idx, dst_idx, n_cores):
    n_src, d_in = feat_src.shape
    n_dst = feat_dst.shape[0]
    assert d_in % 128 == 0
    fch = d_in // 128

    ndc = _cdiv(n_dst, n_cores)                    # dsts per core
    ndc_pad = _cdiv(ndc, PGROUP * W_DST) * PGROUP * W_DST
    nwin = ndc_pad // W_DST
    nsrc_pad = _cdiv(n_src, 128) * 128
    sent_row = nsrc_pad                            # sentinel table row id
    nt_src = nsrc_pad // 128
    nt_dst = ndc_pad // 128

    # ---- edge sort by dst
    perm = np.argsort(dst_idx, kind="stable")
    ds = dst_idx[perm]
    ss = src_idx[perm]

    counts = np.zeros((n_cores, nwin), np.int64)
    per_core = []
    for c in range(n_cores):
        lo, hi = np.searchsorted(ds, [c * ndc, min((c + 1) * ndc, n_dst)])
        d_loc = (ds[lo:hi] - c * ndc).astype(np.int64)
        s_loc = ss[lo:hi].astype(np.int64)
        win = d_loc // W_DST
        counts[c] = np.bincount(win, minlength=nwin)
        per_core.append((d_loc, s_loc, win))

    order = np.argsort(-counts, axis=1, kind="stable")     # [n_cores, nwin]
    sorted_counts = np.take_along_axis(counts, order, axis=1)
    rank_max = sorted_counts.max(axis=0)                   # [nwin]
    t_r = np.maximum(1, _cdiv(rank_max, 128)).astype(np.int64)  # tiles/slotrank
    slot_base = np.concatenate([[0], np.cumsum(t_r)])
    stot = int(slot_base[-1])
    n_chunks = _cdiv(stot, CHUNK)
    stot_pad = n_chunks * CHUNK

    # schedule shared by all cores: slot -> (window rank, tile, ntiles)
    slot_sched = []
    for r in range(nwin):
        for t in range(int(t_r[r])):
            slot_sched.append((r, t, int(t_r[r])))
    assert len(slot_sched) == stot

    # ---- per-core edge slot arrays
    idxs_h, dloc_h = [], []
    for c in range(n_cores):
        d_loc, s_loc, win = per_core[c]
        e_src_ids = np.full((stot_pad, 128), sent_row, np.int32)
        e_dloc = np.zeros((stot_pad, 128), np.float32)
        if len(d_loc):
            rank = np.empty(nwin, np.int64)
            rank[order[c]] = np.arange(nwin)
            win_start = np.concatenate([[0], np.cumsum(counts[c])[:-1]])
            posw = np.arange(len(d_loc)) - win_start[win]
            r_of = rank[win]
            slot = slot_base[r_of] + posw // 128
            lane = posw % 128
            e_src_ids[slot, lane] = s_loc
            e_dloc[slot, lane] = (d_loc - win * W_DST).astype(np.float32)
        # [n_chunks, 128, CHUNK]: arr[i, p, j] = slot i*CHUNK+j, lane p
        def _pack(a):
            return np.ascontiguousarray(
                a.reshape(n_chunks, CHUNK, 128).transpose(0, 2, 1)
            )
        idxs_h.append(_pack(e_src_ids))
        dloc_h.append(_pack(e_dloc))

    # ---- feature tiles, feature-major contiguous: [fch, 128(f), npad(n)]
    def _tiles(feat, npad):
        f = np.zeros((npad, d_in), np.float32)
        f[: feat.shape[0]] = feat
        return np.ascontiguousarray(
            f.reshape(npad, fch, 128).transpose(1, 2, 0)
        ).astype(NP_BF16)

    def _tiles_arr(f):
        return np.ascontiguousarray(
            f.reshape(f.shape[0], fch, 128).transpose(1, 2, 0)
        ).astype(NP_BF16)

    fsT = _tiles(feat_src, nsrc_pad)
    # feat_dst shard rows permuted into slot (sorted-window) order so the
    # e_dst table comes out slot-ordered with compile-time addresses
    fdT = []
    for c in range(n_cores):
        n_here = min(ndc, n_dst - c * ndc)
        fd_slot = np.zeros((ndc_pad, d_in), np.float32)
        for r in range(nwin):
            w = int(order[c][r])
            d0 = w * W_DST
            n = min(W_DST, n_here - d0)
            if n > 0:
                fd_slot[r * W_DST : r * W_DST + n] = \
                    feat_dst[c * ndc + d0 : c * ndc + d0 + n]
        fdT.append(_tiles_arr(fd_slot))

    # ---- attention selector matrices (pure relayout of attn input)
    a_src = np.zeros((HD, H), np.float32)
    a_dst = np.zeros((HD, H), np.float32)
    for h in range(H):
        a_dst[h * D : (h + 1) * D, h] = attn[h, :D]
        a_src[h * D : (h + 1) * D, h] = attn[h, D:]

    cfg = dict(
        n_src=n_src, n_dst=n_dst, d_in=d_in, fch=fch, ndc=ndc,
        ndc_pad=ndc_pad, nwin=nwin, nsrc_pad=nsrc_pad, sent_row=sent_row,
        nt_src=nt_src, nt_dst=nt_dst, stot=stot, stot_pad=stot_pad,
        n_chunks=n_chunks, slot_sched=slot_sched, n_cores=n_cores,
    )
    common = dict(
        wsrc=np.ascontiguousarray(w_src).astype(NP_BF16),
        wsrcT=np.ascontiguousarray(w_src.T).astype(NP_BF16),
        wdstT=np.ascontiguousarray(w_dst.T).astype(NP_BF16),
        asrc=np.ascontiguousarray(a_src).astype(NP_BF16),
        adst=np.ascontiguousarray(a_dst).astype(NP_BF16),
        fsT=fsT,
    )
    in_maps = []
    for c in range(n_cores):
        m = dict(common)
        m["fdT"] = fdT[c]
        m["idxs"] = idxs_h[c]
        m["dloc"] = dloc_h[c]
        in_maps.append(m)
    return cfg, in_maps, order


# ---------------------------------------------------------------- device kernel
def _build(nc, tc, cfg):
    fch = cfg["fch"]
    d_in = cfg["d_in"]
    nt_src = cfg["nt_src"]
    nt_dst = cfg["nt_dst"]

    # I/O
    fsT = nc.dram_tensor("fsT", [fch, 128, cfg["nsrc_pad"]], BF16,
                         kind="ExternalInput")
    fdT = nc.dram_tensor("fdT", [fch, 128, cfg["ndc_pad"]], BF16,
                         kind="ExternalInput")
    wsrc = nc.dram_tensor("wsrc", [d_in, HD], BF16, kind="ExternalInput")
    wsrcT = nc.dram_tensor("wsrcT", [HD, d_in], BF16, kind="ExternalInput")
    wdstT = nc.dram_tensor("wdstT", [HD, d_in], BF16, kind="ExternalInput")
    asrc = nc.dram_tensor("asrc", [HD, H], BF16, kind="ExternalInput")
    adst = nc.dram_tensor("adst", [HD, H], BF16, kind="ExternalInput")
    idxs = nc.dram_tensor("idxs", [cfg["n_chunks"], 128, CHUNK], I32,
                          kind="ExternalInput")
    dloc = nc.dram_tensor("dloc", [cfg["n_chunks"], 128, CHUNK], F32,
                          kind="ExternalInput")
    out = nc.dram_tensor("out", [cfg["ndc_pad"], HD], F32,
                         kind="ExternalOutput")

    tab = nc.dram_tensor("tab", [cfg["nsrc_pad"] + 128, TCB], BF16,
                         kind="Internal")
    edt = nc.dram_tensor("edt", [cfg["ndc_pad"], H], BF16, kind="Internal")

    import contextlib
    ctx = contextlib.ExitStack()
    with ctx:
        const = ctx.enter_context(tc.tile_pool(name="const", bufs=1))
        sb = ctx.enter_context(tc.tile_pool(name="sb", bufs=3))
        gp = ctx.enter_context(tc.tile_pool(name="gp", bufs=4))
        pp = ctx.enter_context(tc.tile_pool(name="pp", bufs=3, space="PSUM"))

        # ---------------- setup: W_ext = [w_src | M_src], M_dst
        wsT_sb = const.tile([128, d_in], BF16, tag="wsT")
        wdT_sb = const.tile([128, d_in], BF16, tag="wdT")
        asrc_sb = const.tile([128, H], BF16, tag="asrc")
        adst_sb = const.tile([128, H], BF16, tag="adst")
        nc.sync.dma_start(out=wsT_sb[:], in_=wsrcT[:, :])
        nc.sync.dma_start(out=wdT_sb[:], in_=wdstT[:, :])
        nc.sync.dma_start(out=asrc_sb[:], in_=asrc[:, :])
        nc.sync.dma_start(out=adst_sb[:], in_=adst[:, :])

        wext = []
        mdst = []
        for k in range(fch):
            we = const.tile([128, TC], BF16, tag=f"wext{k}")
            nc.sync.dma_start(out=we[:, :HD],
                              in_=wsrc[k * 128 : (k + 1) * 128, :])
            pm = pp.tile([128, 512], F32, tag="acc")
            nc.tensor.matmul(pm[:, :H], wsT_sb[:, k * 128 : (k + 1) * 128],
                             asrc_sb[:], start=True, stop=True)
            nc.vector.tensor_copy(we[:, HD:TC], pm[:, :H])
            wext.append(we)

            md = const.tile([128, H], BF16, tag=f"mdst{k}")
            pm2 = pp.tile([128, 512], F32, tag="acc")
            nc.tensor.matmul(pm2[:, :H], wdT_sb[:, k * 128 : (k + 1) * 128],
                             adst_sb[:], start=True, stop=True)
            nc.vector.tensor_copy(md[:], pm2[:, :H])
            mdst.append(md)

        iota_i = const.tile([128, W_DST], I32, tag="iota_i")
        iota_f = const.tile([128, W_DST], F32, tag="iota_f")
        nc.gpsimd.iota(iota_i[:], pattern=[[1, W_DST]], base=0,
                       channel_multiplier=0)
        nc.vector.tensor_copy(iota_f[:], iota_i[:])

        from concourse.masks import make_identity
        ident = const.tile([128, 128], BF16, tag="ident")
        make_identity(nc, ident[:])

        # ---------------- phase B: e_dst table (slot-rank order)
        for jb in range(0, nt_dst, TB):
            tcur = min(TB, nt_dst - jb)
            lh = sb.tile([128, fch * TB * 128], BF16, tag="lhb")
            lh3 = lh[:].rearrange("p (k m) -> p k m", k=fch)
            for k in range(fch):
                nc.sync.dma_start(
                    out=lh3[:, k, : tcur * 128],
                    in_=fdT[k, :, jb * 128 : (jb + tcur) * 128])
            ebo = sb.tile([128, TB * H], BF16, tag="ebo")
            for t in range(tcur):
                pb = pp.tile([128, 512], F32, tag="acc")
                for k in range(fch):
                    nc.tensor.matmul(
                        pb[:, :H],
                        lh3[:, k, t * 128 : (t + 1) * 128],
                        mdst[k][:], start=(k == 0), stop=(k == fch - 1))
                nc.vector.tensor_copy(ebo[:, t * H : (t + 1) * H], pb[:, :H])
            nc.sync.dma_start(
                out=edt[jb * 128 : (jb + tcur) * 128, :].rearrange(
                    "(t p) c -> p t c", p=128),
                in_=ebo[:].rearrange("p (t c) -> p t c", c=H)[:, :tcur])

        # ---------------- phase C: gather table [fs bf16 | e_src f32]
        for jb in range(0, nt_src, TB):
            tcur = min(TB, nt_src - jb)
            lh = sb.tile([128, fch * TB * 128], BF16, tag="lhc")
            lh3 = lh[:].rearrange("p (k m) -> p k m", k=fch)
            for k in range(fch):
                nc.sync.dma_start(
                    out=lh3[:, k, : tcur * 128],
                    in_=fsT[k, :, jb * 128 : (jb + tcur) * 128])
            tbo = sb.tile([128, TB * TCB], BF16, tag="tbo")
            tbo_f = tbo[:].bitcast(F32).rearrange("p (t c) -> p t c",
                                                  c=TCB // 2)
            tbo_b = tbo[:].rearrange("p (t c) -> p t c", c=TCB)
            for t in range(tcur):
                pc = pp.tile([128, 512], F32, tag="acc")
                for k in range(fch):
                    nc.tensor.matmul(
                        pc[:, :TC],
                        lh3[:, k, t * 128 : (t + 1) * 128],
                        wext[k][:], start=(k == 0), stop=(k == fch - 1))
                nc.vector.tensor_copy(tbo_b[:, t, :HD], pc[:, :HD])
                nc.vector.tensor_copy(tbo_f[:, t, HD // 2 : HD // 2 + H],
                                      pc[:, HD:TC])
            nc.sync.dma_start(
                out=tab[jb * 128 : (jb + tcur) * 128, :].rearrange(
                    "(t p) c -> p t c", p=128),
                in_=tbo_b[:, :tcur])

        # sentinel rows (aligned block of 128): fs = 0, e_src = SENT_ESRC
        st = sb.tile([128, TCB], BF16, tag="sent")
        nc.vector.memset(st[:, :HD], 0.0)
        nc.vector.memset(st[:].bitcast(F32)[:, HD // 2 : HD // 2 + H],
                         SENT_ESRC)
        nc.sync.dma_start(
            out=tab[cfg["nsrc_pad"] : cfg["nsrc_pad"] + 128, :].rearrange(
                "(t p) c -> p t c", p=128),
            in_=st[:].rearrange("p (t c) -> p t c", t=1))

        # preload all chunk indices / window positions once
        n_chunks = cfg["n_chunks"]
        ixs_all = const.tile([128, n_chunks * CHUNK], I32, tag="ixsall")
        dl_all = const.tile([128, n_chunks * CHUNK], F32, tag="dlall")
        nc.sync.dma_start(
            out=ixs_all[:].rearrange("p (i c) -> p i c", c=CHUNK),
            in_=idxs[:, :, :].rearrange("i p c -> p i c"))
        nc.sync.dma_start(
            out=dl_all[:].rearrange("p (i c) -> p i c", c=CHUNK),
            in_=dloc[:, :, :].rearrange("i p c -> p i c"))

        # ---------------- main pass
        sched = cfg["slot_sched"]
        psg = {}
        ewin = {}
        for i in range(cfg["n_chunks"]):
            gt = gp.tile([128, CHUNK * TCB], BF16, tag="gt", bufs=10)
            ixs = ixs_all[:, i * CHUNK : (i + 1) * CHUNK]
            dl = dl_all[:, i * CHUNK : (i + 1) * CHUNK]

            gt3 = gt[:].rearrange("p (s c) -> p s c", c=TCB)
            esv = gt[:].bitcast(F32).rearrange(
                "p (s c) -> p s c", c=TCB // 2)[:, :, HD // 2 : HD // 2 + H]
            # per-slot indirect gathers: one 272B row offset per partition
            for sl in range(CHUNK):
                if i * CHUNK + sl >= cfg["stot"]:
                    break
                nc.gpsimd.indirect_dma_start(
                    out=gt[:, sl * TCB : (sl + 1) * TCB], out_offset=None,
                    in_=tab[:, :],
                    in_offset=IndirectOffsetOnAxis(ap=ixs[:, sl : sl + 1],
                                                   axis=0))

            # indicator S: [128, CHUNK * W_DST] in bf16
            sbt = sb.tile([128, CHUNK * W_DST], BF16, tag="sbt", bufs=8)
            nc.vector.tensor_tensor(
                out=sbt[:].rearrange("p (s w) -> p s w", w=W_DST),
                in0=iota_f[:].rearrange("p (o w) -> p o w", o=1).to_broadcast(
                    [128, CHUNK, W_DST]),
                in1=dl.rearrange("p (s o) -> p s o", o=1).to_broadcast(
                    [128, CHUNK, W_DST]),
                op=mybir.AluOpType.is_equal)

            # per-edge e_dst via PE: transpose indicator, multiply by the
            # window's e_dst rows; all slots accumulate into one PSUM tile
            peb = pp.tile([128, 512], F32, tag="peb", name=f"peb{i}", bufs=3)
            for sl in range(CHUNK):
                s = i * CHUNK + sl
                if s >= cfg["stot"]:
                    break
                r, t, tr = sched[s]
                if t == 0 and r not in ewin:
                    ew = sb.tile([W_DST, H], BF16, tag="ewin", name=f"ew{r}")
                    nc.sync.dma_start(
                        out=ew[:], in_=edt[r * W_DST : (r + 1) * W_DST, :])
                    ewin[r] = ew
                ptr = pp.tile([W_DST, 128], BF16, tag="tr", name=f"ptr{s}",
                              bufs=2)
                nc.tensor.transpose(ptr[:],
                                    sbt[:, sl * W_DST : (sl + 1) * W_DST],
                                    ident[:])
                stx = sb.tile([W_DST, 128], BF16, tag="stx")
                nc.vector.tensor_copy(stx[:], ptr[:])
                nc.tensor.matmul(peb[:, sl * H : (sl + 1) * H], stx[:],
                                 ewin[r][:], start=True, stop=True)
                if t == tr - 1:
                    ewin.pop(r, None)

            # logits: e = e_src + e_dst, leaky-relu, exp (f32, in est)
            ns = min(CHUNK, cfg["stot"] - i * CHUNK)
            est = sb.tile([128, CHUNK * H], F32, tag="est")
            est3 = est[:].rearrange("p (s c) -> p s c", c=H)
            nc.vector.tensor_tensor(
                out=est3[:, :ns], in0=esv[:, :ns],
                in1=peb[:, : ns * H].rearrange("p (s c) -> p s c", c=H),
                op=mybir.AluOpType.add)
            tmp = sb.tile([128, CHUNK * H], F32, tag="tmp")
            nc.vector.tensor_scalar(out=tmp[:, : ns * H],
                                    in0=est[:, : ns * H],
                                    scalar1=NEG_SLOPE,
                                    scalar2=None, op0=mybir.AluOpType.mult)
            nc.vector.tensor_tensor(out=est[:, : ns * H],
                                    in0=est[:, : ns * H],
                                    in1=tmp[:, : ns * H],
                                    op=mybir.AluOpType.max)
            nc.scalar.activation(est[:, : ns * H], est[:, : ns * H],
                                 mybir.ActivationFunctionType.Exp)
            # cast ex back into the bf16 edge rows (cols HD..HD+H)
            nc.vector.tensor_copy(gt3[:, :ns, HD : HD + H], est3[:, :ns])

            # scale fs columns by per-head ex
            for h in range(H):
                fv = gt3[:, :ns, h * D : (h + 1) * D]
                xv = gt3[:, :ns, HD + h : HD + h + 1].to_broadcast(
                    [128, ns, D])
                nc.vector.tensor_tensor(out=fv, in0=fv, in1=xv,
                                        op=mybir.AluOpType.mult)

            # segment matmuls
            for sl in range(CHUNK):
                s = i * CHUNK + sl
                if s >= cfg["stot"]:
                    break
                r, t, tr = sched[s]
                g, q = r // PGROUP, r % PGROUP
                if q == 0 and t == 0:
                    psg[g] = pp.tile([128, 512], F32, tag="acc",
                                     name=f"psg{g}")
                nc.tensor.matmul(
                    psg[g][q * W_DST : (q + 1) * W_DST, :TC],
                    sbt[:, sl * W_DST : (sl + 1) * W_DST],
                    gt[:, sl * TCB : sl * TCB + TC],
                    start=(t == 0), stop=(t == tr - 1),
                    tile_position=(0, q * W_DST))
                if q == PGROUP - 1 and t == tr - 1:
                    # epilogue for group g
                    pt = psg.pop(g)
                    dmx = sb.tile([128, H], F32, tag="dmx")
                    rcp = sb.tile([128, H], F32, tag="rcp")
                    nc.vector.tensor_scalar(out=dmx[:], in0=pt[:, HD:TC],
                                            scalar1=1e-30, scalar2=None,
                                            op0=mybir.AluOpType.max)
                    nc.vector.reciprocal(rcp[:], dmx[:])
                    ot = sb.tile([128, HD], F32, tag="ot")
                    for h in range(H):
                        nc.vector.tensor_scalar(
                            out=ot[:, h * D : (h + 1) * D],
                            in0=pt[:, h * D : (h + 1) * D],
                            scalar1=rcp[:, h : h + 1], scalar2=0.0,
                            op0=mybir.AluOpType.mult,
                            op1=mybir.AluOpType.max)
                    nc.sync.dma_start(
                        out=out[g * 128 : (g + 1) * 128, :], in_=ot[:])
    return out


# ---------------------------------------------------------------- entry point
def kernel(feat_src, feat_dst, w_src, w_dst, attn, src_idx, dst_idx,
           _n_cores=N_CORES, _backend="hw", _results_hook=None,
           _runner=None):
    feat_src = np.asarray(feat_src, np.float32)
    feat_dst = np.asarray(feat_dst, np.float32)
    w_src = np.asarray(w_src, np.float32)
    w_dst = np.asarray(w_dst, np.float32)
    attn = np.asarray(attn, np.float32)
    src_idx = np.asarray(src_idx).astype(np.int32)
    dst_idx = np.asarray(dst_idx).astype(np.int32)

    cfg, in_maps, order = _prep(feat_src, feat_dst, w_src, w_dst, attn,
                                src_idx, dst_idx, _n_cores)

    nc = bacc.Bacc("TRN2", target_bir_lowering=False, debug=False)
    with tile.TileContext(nc) as tc:
        _build(nc, tc, cfg)
    nc.compile()

    if _backend == "sim":
        from concourse.bass_interp import CoreSim
        results = []
        for c in range(_n_cores):
            sim = CoreSim(nc, trace=False, require_nnan=False,
                          require_finite=False)
            for name, arr in in_maps[c].items():
                sim.tensor(name)[:] = arr
            sim.simulate(check_with_hw=False)
            results.append({"out": np.array(sim.tensor("out"))})
        res_obj = None
    elif _runner is not None:
        results = _runner(nc, in_maps)
        res_obj = None
    else:
        res_obj = run_bass_kernel_spmd(nc, in_maps,
                                       core_ids=list(range(_n_cores)))
        results = res_obj.results
    if _results_hook is not None:
        _results_hook(res_obj)

    # unpermute slot-ordered outputs back to dst ids
    n_dst = cfg["n_dst"]
    ndc = cfg["ndc"]
    out_full = np.zeros((n_dst, HD), np.float32)
    for c in range(_n_cores):
        oc = results[c]["out"].reshape(cfg["nwin"], W_DST, HD)
        n_here = min(ndc, n_dst - c * ndc)
        for r in range(cfg["nwin"]):
            w = int(order[c][r])
            d0 = w * W_DST
            n = min(W_DST, n_here - d0)
            if n > 0:
                out_full[c * ndc + d0 : c * ndc + d0 + n] = oc[r, :n]
    return out_full
